# Initial kernel scaffold
#
"""Trainium2 Bass kernel for nn_DepthCalibration.

Math (per batch b):
  s      = conv1d(pred*g, w, pad=1) + cb                     (smoothed depths)
  e[n,m] = -2*||ray_n - ray_m||^2                            (sigma=0.5 fixed)
  out[n] = clip(sum_m exp(e[n,m]) * s[m], 0.1, 100)

Strategy: one batch per NeuronCore (B=8, 8 cores, fully data parallel).
The exponent is a rank-9 augmented inner product:
  e = 4*r.r' - 2(x^2+y^2+z^2) - 2(x'^2+y'^2+z'^2)
    = matmul(A[:,n], B[:,m])  with
  A = [x, y, z, x^2, y^2, z^2, 1, 1, 1]   (stationary side, f32r)
  B = [4x', 4y', 4z', -2, -2, -2, -2x'^2, -2y'^2, -2z'^2]
so the TensorEngine produces exp-arguments directly into PSUM (f32r runs
at 1 cycle/row vs fp32's 4x-slow path; measured 1.9e-4 matmul accuracy);
ScalarE exp (the 1 elem/lane/cycle floor, ~109us/core) converts to fp16
weights; the weighted row-sum is one fused DVE scalar_tensor_tensor per
128-row block (products computed in fp32 internally, accumulated to a
f32 [128,1] column) against a partition-broadcast copy of s.

Engine budget per core (measured): ACT exp ~121us, DVE mv ~146us,
PE matmuls ~62us, all overlapped; steady-state ~225us/call end to end.
The conv1d smoothing, augmented-matrix construction, and clipping all
run on device; the host only reshapes/pads/transposes inputs for DMA
friendliness (strided 4-byte DRAM reads are descriptor-dominated).
"""

import sys
import os

sys.path.insert(0, "/opt/trn_rl_repo")

import numpy as np

from concourse import bass, mybir
from concourse import bacc
from concourse import tile
from concourse.bass_utils import run_bass_kernel_spmd

B, N = 8, 4096
NB = N // 128          # 32 row blocks of 128
CHUNK = 2048           # ACT chunk (4 PSUM banks)
NCHUNK = N // CHUNK    # 2 chunks per row block
MM = 512               # matmul moving free dim (one PSUM bank of fp32)
MIN_DEPTH, MAX_DEPTH = 0.1, 100.0

F32 = mybir.dt.float32
F32R = mybir.dt.float32r
FP16 = mybir.dt.float16

KAUG = 9               # augmented contraction depth
ALT = True             # alternate PE row groups to hide LDWEIGHTS
WIDE_STT = True        # one [128, N] STT per row block vs per-chunk
WBUFS = 3              # exp-output (W) tile buffers
SCBUFS = 2             # STT scratch-output buffers
SKIP_STT = False       # ablation: drop the DVE weighted-sum
SKIP_EXP = False       # ablation: drop the ACT exp
SKIP_MM = False        # ablation: drop the matmuls
SKIP_PREP_AB = False   # ablation: drop A/B aug build
SKIP_PREP_S = False    # ablation: drop s conv/broadcast chain


def build_program(gw0, gw1, gw2, cb, w_dtype=FP16, repeat=1):
    """Build the single-core program (run SPMD on 8 cores).

    gw0/gw1/gw2: conv taps pre-multiplied by global_scale; cb: conv bias.
    repeat>1 wraps the body in a hardware loop (for timing measurement).
    """
    nc = bacc.Bacc(
        "TRN2",
        target_bir_lowering=False,
        debug=False,
        enable_asserts=False,
        num_devices=8,
    )

    pred_pad = nc.dram_tensor("pred_pad", (N + 2,), F32, kind="ExternalInput").ap()
    rayT = nc.dram_tensor("rayT", (3, N), F32, kind="ExternalInput").ap()
    out = nc.dram_tensor("out", (N,), F32, kind="ExternalOutput").ap()
    s_dram = nc.dram_tensor("s_scratch", (N,), w_dtype, kind="Internal").ap()

    AF = mybir.ActivationFunctionType
    OP = mybir.AluOpType

    from contextlib import ExitStack

    ngrp = 2 if ALT else 1

    with tile.TileContext(nc) as tc, ExitStack() as stk:
        if repeat > 1:
            ET = mybir.EngineType
            stk.enter_context(
                tc.For_i(
                    0,
                    repeat,
                    1,
                    hint_engines=(ET.PE, ET.DVE, ET.Activation, ET.SP, ET.Pool),
                )
            )
        with (
            tc.tile_pool(name="const", bufs=1) as cpool,
            tc.tile_pool(name="w", bufs=WBUFS) as wpool,
            tc.tile_pool(name="ttr", bufs=SCBUFS) as tpool,
            tc.tile_pool(name="psum", bufs=2, space="PSUM") as ppool,
        ):
            # ---------------- aug matrices A (stationary) and B (moving) ----
            # duplicated at base partition 32 so consecutive row blocks use
            # different PE row groups (LDWEIGHTS overlaps in-flight matmuls)
            A = cpool.tile([32 * (ngrp - 1) + KAUG, N], F32R)
            Bm = cpool.tile([32 * (ngrp - 1) + KAUG, N], F32R)
            R = cpool.tile([3, N], F32)      # raw rays (x,y,z rows)
            sqm = cpool.tile([3, N], F32R)   # -2x^2 ...
            r4 = cpool.tile([3, N], F32R)    # 4x ...
            ones3 = nc.inline_tensor(np.ones((3, N), np.float32), "ones3").ap()
            m2s3 = nc.inline_tensor(np.full((3, N), -2.0, np.float32), "m2s3").ap()

            if not SKIP_PREP_AB:
                # A = [r, r^2, -2*1s]; B = [4r', -2*1s, r'^2]
                # squares/copy on ACT (idle at prep); only r4 stays on DVE
                nc.sync.dma_start(R[:], rayT[:, :])
                nc.scalar.activation(A[0:3, :], R[:], AF.Identity)
                nc.scalar.activation(sqm[:], R[:], AF.Square)
                nc.vector.tensor_scalar_mul(r4[:], R[:], 4.0)
                nc.sync.dma_start(A[3:6, :], sqm[:])
                nc.sync.dma_start(A[6:9, :], m2s3.bitcast(F32R))
                nc.sync.dma_start(Bm[0:3, :], r4[:])
                nc.sync.dma_start(Bm[6:9, :], sqm[:])
                nc.sync.dma_start(Bm[3:6, :], m2s3.bitcast(F32R))
                for g in range(1, ngrp):
                    nc.sync.dma_start(A[32 * g : 32 * g + KAUG, :], A[0:KAUG, :])
                    nc.sync.dma_start(Bm[32 * g : 32 * g + KAUG, :], Bm[0:KAUG, :])

            # ---------------- smoothed depths s (vertical layout) -----------
            # V*[p, c] = pred_pad[off + p + 128c];  s[i] for i = p + 128c
            def vload(off):
                t = cpool.tile([128, NB], F32, tag=f"v{off}")
                src = pred_pad[off : off + N].rearrange("(c p) -> p c", p=128)
                nc.sync.dma_start(t[:], src)
                return t

            sv = cpool.tile([128, NB], F32)
            vl, vc, vr = vload(0), vload(1), vload(2)
            if SKIP_PREP_S:
                nc.vector.memset(sv[:], 0.5)
            if not SKIP_PREP_S:
                nc.vector.tensor_scalar_mul(sv[:], vl[:], gw0)
                nc.vector.scalar_tensor_tensor(
                    sv[:], vc[:], gw1, sv[:], OP.mult, OP.add
                )
                nc.vector.scalar_tensor_tensor(
                    sv[:], vr[:], gw2, sv[:], OP.mult, OP.add
                )
                nc.vector.tensor_scalar_add(sv[:], sv[:], cb)
            sv_c = cpool.tile([128, NB], w_dtype)
            nc.vector.tensor_copy(sv_c[:], sv[:])
            # to DRAM (linear: i = p + 128c) and broadcast to 128 partitions
            nc.sync.dma_start(s_dram.rearrange("(c p) -> p c", p=128), sv_c[:])
            s_bc = cpool.tile([128, N], w_dtype)
            for q in range(4):
                sl = slice(q * (N // 4), (q + 1) * (N // 4))
                nc.sync.dma_start(
                    s_bc[:, sl],
                    s_dram[sl].rearrange("(o n) -> o n", o=1).broadcast_to(
                        (128, N // 4)
                    ),
                )

            # ---------------- main loop ------------------------------------
            acc = cpool.tile([128, NB], F32)  # per-row-block accumulators
            accp = cpool.tile([128, NB * NCHUNK], F32)  # per-chunk partials
            for i in range(NB):
                g = 32 * (i % ngrp)
                lhsT = A[g : g + KAUG, i * 128 : (i + 1) * 128]
                if WIDE_STT:
                    wt = wpool.tile([128, N], w_dtype, tag="w")
                    for c in range(NCHUNK):
                        m0 = c * CHUNK
                        pt = ppool.tile([128, CHUNK], F32, tag="ps")
                        if not SKIP_MM:
                            for j in range(CHUNK // MM):
                                nc.tensor.matmul(
                                    pt[:, j * MM : (j + 1) * MM],
                                    lhsT,
                                    Bm[g : g + KAUG, m0 + j * MM : m0 + (j + 1) * MM],
                                )
                        if not SKIP_EXP:
                            nc.scalar.activation(wt[:, m0 : m0 + CHUNK], pt[:], AF.Exp)
                        else:
                            nc.vector.memset(wt[0:1, m0 : m0 + 2], 0.5)
                    if not SKIP_STT:
                        sc = tpool.tile([128, N], w_dtype, tag="sc")
                        nc.vector.scalar_tensor_tensor(
                            sc[:],
                            wt[:],
                            0.0,
                            s_bc[:],
                            OP.bypass,
                            OP.mult,
                            accum_out=acc[:, i : i + 1],
                        )
                    else:
                        nc.vector.memset(acc[:, i : i + 1], 0.5)
                else:
                    acc2 = acc  # per-chunk partials combined below
                    for c in range(NCHUNK):
                        m0 = c * CHUNK
                        pt = ppool.tile([128, CHUNK], F32, tag="ps")
                        for j in range(CHUNK // MM):
                            nc.tensor.matmul(
                                pt[:, j * MM : (j + 1) * MM],
                                lhsT,
                                Bm[g : g + KAUG, m0 + j * MM : m0 + (j + 1) * MM],
                            )
                        wt = wpool.tile([128, CHUNK], w_dtype, tag="w")
                        nc.scalar.activation(wt[:], pt[:], AF.Exp)
                        sc = tpool.tile([128, CHUNK], w_dtype, tag="sc")
                        nc.vector.scalar_tensor_tensor(
                            sc[:],
                            wt[:],
                            0.0,
                            s_bc[:, m0 : m0 + CHUNK],
                            OP.bypass,
                            OP.mult,
                            accum_out=accp[:, i * NCHUNK + c : i * NCHUNK + c + 1],
                        )

            # ---------------- clip + store ---------------------------------
            if not WIDE_STT:
                nc.vector.tensor_add(acc[:], accp[:, 0::NCHUNK], accp[:, 1::NCHUNK])
            res = cpool.tile([128, NB], F32)
            nc.vector.tensor_scalar(
                res[:],
                acc[:],
                MIN_DEPTH,
                MAX_DEPTH,
                OP.max,
                OP.min,
            )
            nc.sync.dma_start(out.rearrange("(i p) -> p i", p=128), res[:])

    nc.compile()
    return nc


_cache = {}


def _get_program(key, gw0, gw1, gw2, cb, w_dtype, repeat=1):
    key = key + (repeat,)
    if key not in _cache:
        _cache[key] = build_program(gw0, gw1, gw2, cb, w_dtype, repeat=repeat)
    return _cache[key]


def kernel(pred_depth, ray_3d, conv_w, conv_b, global_scale, repeat=1):
    pred_depth = np.asarray(pred_depth, np.float32)
    ray_3d = np.asarray(ray_3d, np.float32)
    g = float(np.asarray(global_scale).reshape(-1)[0])
    w = np.asarray(conv_w, np.float32).reshape(-1)
    cb = float(np.asarray(conv_b).reshape(-1)[0])
    gw0, gw1, gw2 = float(w[0] * g), float(w[1] * g), float(w[2] * g)

    nc = _get_program((gw0, gw1, gw2, cb), gw0, gw1, gw2, cb, FP16, repeat=repeat)

    in_maps = []
    for b in range(B):
        pp = np.zeros(N + 2, np.float32)
        pp[1 : N + 1] = pred_depth[b]
        in_maps.append(
            {
                "pred_pad": pp,
                "rayT": np.ascontiguousarray(ray_3d[b].T),
            }
        )
    res = _run_with_retry(nc, in_maps)
    out = np.stack([res.results[b]["out"] for b in range(B)]).astype(np.float32)
    return out


def _run_with_retry(nc, in_maps, tries=3):
    # The shared axon device occasionally reports a transient
    # NRT_EXEC_UNIT_UNRECOVERABLE after a prior process crashed; it
    # recovers within ~20s. Retry rather than failing the whole call.
    import time as _time

    for attempt in range(tries):
        try:
            return run_bass_kernel_spmd(nc, in_maps, core_ids=list(range(B)))
        except Exception:
            if attempt == tries - 1:
                raise
            _time.sleep(25)



# revision 1
# speedup vs baseline: 1.3618x; 1.3618x over previous
"""Trainium2 Bass kernel for nn_DepthCalibration.

Math (per batch b):
  s      = conv1d(pred*g, w, pad=1) + cb                     (smoothed depths)
  e[n,m] = -2*||ray_n - ray_m||^2                            (sigma=0.5 fixed)
  out[n] = clip(sum_m exp(e[n,m]) * s[m], 0.1, 100)

Strategy: one batch per NeuronCore (B=8, 8 cores, fully data parallel).
The exponent is a rank-9 augmented inner product:
  e = 4*r.r' - 2(x^2+y^2+z^2) - 2(x'^2+y'^2+z'^2)
    = matmul(A[:,n], B[:,m])  with
  A = [x, y, z, x^2, y^2, z^2, 1, 1, 1]   (stationary side, f32r)
  B = [4x', 4y', 4z', -2, -2, -2, -2x'^2, -2y'^2, -2z'^2]
so the TensorEngine produces exp-arguments directly into PSUM (f32r runs
at 1 cycle/row vs fp32's 4x-slow path; measured 1.9e-4 matmul accuracy);
ScalarE exp (the 1 elem/lane/cycle floor, ~109us/core) converts to fp16
weights; the weighted row-sum is one fused DVE scalar_tensor_tensor per
128-row block (products computed in fp32 internally, accumulated to a
f32 [128,1] column) against a partition-broadcast copy of s.

Engine budget per core (measured): ACT exp ~121us, DVE mv ~146us,
PE matmuls ~62us, all overlapped; steady-state ~225us/call end to end.
The conv1d smoothing, augmented-matrix construction, and clipping all
run on device; the host only reshapes/pads/transposes inputs for DMA
friendliness (strided 4-byte DRAM reads are descriptor-dominated).
"""

import sys
import os

sys.path.insert(0, "/opt/trn_rl_repo")

import numpy as np

from concourse import bass, mybir
from concourse import bacc
from concourse import tile
from concourse.bass_utils import run_bass_kernel_spmd

B, N = 8, 4096
NB = N // 128          # 32 row blocks of 128
CHUNK = 2048           # ACT chunk (4 PSUM banks)
NCHUNK = N // CHUNK    # 2 chunks per row block
MM = 512               # matmul moving free dim (one PSUM bank of fp32)
MIN_DEPTH, MAX_DEPTH = 0.1, 100.0

F32 = mybir.dt.float32
F32R = mybir.dt.float32r
FP16 = mybir.dt.float16

KAUG = 9               # augmented contraction depth
ALT = True             # alternate PE row groups to hide LDWEIGHTS
WIDE_STT = True        # one [128, N] STT per row block vs per-chunk
WBUFS = 3              # exp-output (W) tile buffers
SCBUFS = 2             # STT scratch-output buffers
SKIP_STT = False       # ablation: drop the DVE weighted-sum
SKIP_EXP = False       # ablation: drop the ACT exp
SKIP_MM = False        # ablation: drop the matmuls
SKIP_PREP_AB = False   # ablation: drop A/B aug build
SKIP_PREP_S = False    # ablation: drop s conv/broadcast chain


def build_program(gw0, gw1, gw2, cb, w_dtype=FP16, repeat=1):
    """Build the single-core program (run SPMD on 8 cores).

    gw0/gw1/gw2: conv taps pre-multiplied by global_scale; cb: conv bias.
    repeat>1 wraps the body in a hardware loop (for timing measurement).
    """
    nc = bacc.Bacc(
        "TRN2",
        target_bir_lowering=False,
        debug=False,
        enable_asserts=False,
        num_devices=8,
    )

    pred_pad = nc.dram_tensor("pred_pad", (N + 2,), F32, kind="ExternalInput").ap()
    rayT = nc.dram_tensor("rayT", (3, N), F32, kind="ExternalInput").ap()
    out = nc.dram_tensor("out", (N,), F32, kind="ExternalOutput").ap()
    s_dram = nc.dram_tensor("s_scratch", (N,), w_dtype, kind="Internal").ap()

    AF = mybir.ActivationFunctionType
    OP = mybir.AluOpType

    from contextlib import ExitStack

    ngrp = 2 if ALT else 1

    with tile.TileContext(nc) as tc, ExitStack() as stk:
        if repeat > 1:
            ET = mybir.EngineType
            stk.enter_context(
                tc.For_i(
                    0,
                    repeat,
                    1,
                    hint_engines=(ET.PE, ET.DVE, ET.Activation, ET.SP, ET.Pool),
                )
            )
        with (
            tc.tile_pool(name="const", bufs=1) as cpool,
            tc.tile_pool(name="w", bufs=WBUFS) as wpool,
            tc.tile_pool(name="ttr", bufs=SCBUFS) as tpool,
            tc.tile_pool(name="psum", bufs=2, space="PSUM") as ppool,
        ):
            # ---------------- aug matrices A (stationary) and B (moving) ----
            # duplicated at base partition 32 so consecutive row blocks use
            # different PE row groups (LDWEIGHTS overlaps in-flight matmuls)
            A = cpool.tile([32 * (ngrp - 1) + KAUG, N], F32R)
            Bm = cpool.tile([32 * (ngrp - 1) + KAUG, N], F32R)
            R = cpool.tile([3, N], F32)      # raw rays (x,y,z rows)
            sqm = cpool.tile([3, N], F32R)   # -2x^2 ...
            r4 = cpool.tile([3, N], F32R)    # 4x ...
            ones3 = nc.inline_tensor(np.ones((3, N), np.float32), "ones3").ap()
            m2s3 = nc.inline_tensor(np.full((3, N), -2.0, np.float32), "m2s3").ap()

            if not SKIP_PREP_AB:
                # A = [r, r^2, -2*1s]; B = [4r', -2*1s, r'^2]
                # squares/copy on ACT (idle at prep); only r4 stays on DVE
                nc.sync.dma_start(R[:], rayT[:, :])
                nc.scalar.activation(A[0:3, :], R[:], AF.Identity)
                nc.scalar.activation(sqm[:], R[:], AF.Square)
                nc.vector.tensor_scalar_mul(r4[:], R[:], 4.0)
                nc.sync.dma_start(A[3:6, :], sqm[:])
                nc.sync.dma_start(A[6:9, :], m2s3.bitcast(F32R))
                nc.sync.dma_start(Bm[0:3, :], r4[:])
                nc.sync.dma_start(Bm[6:9, :], sqm[:])
                nc.sync.dma_start(Bm[3:6, :], m2s3.bitcast(F32R))
                for g in range(1, ngrp):
                    nc.sync.dma_start(A[32 * g : 32 * g + KAUG, :], A[0:KAUG, :])
                    nc.sync.dma_start(Bm[32 * g : 32 * g + KAUG, :], Bm[0:KAUG, :])

            # ---------------- smoothed depths s (vertical layout) -----------
            # V*[p, c] = pred_pad[off + p + 128c];  s[i] for i = p + 128c
            def vload(off):
                t = cpool.tile([128, NB], F32, tag=f"v{off}")
                src = pred_pad[off : off + N].rearrange("(c p) -> p c", p=128)
                nc.sync.dma_start(t[:], src)
                return t

            sv = cpool.tile([128, NB], F32)
            vl, vc, vr = vload(0), vload(1), vload(2)
            if SKIP_PREP_S:
                nc.vector.memset(sv[:], 0.5)
            if not SKIP_PREP_S:
                nc.vector.tensor_scalar_mul(sv[:], vl[:], gw0)
                nc.vector.scalar_tensor_tensor(
                    sv[:], vc[:], gw1, sv[:], OP.mult, OP.add
                )
                nc.vector.scalar_tensor_tensor(
                    sv[:], vr[:], gw2, sv[:], OP.mult, OP.add
                )
                nc.vector.tensor_scalar_add(sv[:], sv[:], cb)
            sv_c = cpool.tile([128, NB], w_dtype)
            nc.vector.tensor_copy(sv_c[:], sv[:])
            # to DRAM (linear: i = p + 128c) and broadcast to 128 partitions
            nc.sync.dma_start(s_dram.rearrange("(c p) -> p c", p=128), sv_c[:])
            s_bc = cpool.tile([128, N], w_dtype)
            for q in range(4):
                sl = slice(q * (N // 4), (q + 1) * (N // 4))
                nc.sync.dma_start(
                    s_bc[:, sl],
                    s_dram[sl].rearrange("(o n) -> o n", o=1).broadcast_to(
                        (128, N // 4)
                    ),
                )

            # ---------------- main loop ------------------------------------
            acc = cpool.tile([128, NB], F32)  # per-row-block accumulators
            accp = cpool.tile([128, NB * NCHUNK], F32)  # per-chunk partials
            for i in range(NB):
                g = 32 * (i % ngrp)
                lhsT = A[g : g + KAUG, i * 128 : (i + 1) * 128]
                if WIDE_STT:
                    wt = wpool.tile([128, N], w_dtype, tag="w")
                    for c in range(NCHUNK):
                        m0 = c * CHUNK
                        pt = ppool.tile([128, CHUNK], F32, tag="ps")
                        if not SKIP_MM:
                            for j in range(CHUNK // MM):
                                nc.tensor.matmul(
                                    pt[:, j * MM : (j + 1) * MM],
                                    lhsT,
                                    Bm[g : g + KAUG, m0 + j * MM : m0 + (j + 1) * MM],
                                )
                        if not SKIP_EXP:
                            nc.scalar.activation(wt[:, m0 : m0 + CHUNK], pt[:], AF.Exp)
                        else:
                            nc.vector.memset(wt[0:1, m0 : m0 + 2], 0.5)
                    if not SKIP_STT:
                        sc = tpool.tile([128, N], w_dtype, tag="sc")
                        nc.vector.scalar_tensor_tensor(
                            sc[:],
                            wt[:],
                            0.0,
                            s_bc[:],
                            OP.bypass,
                            OP.mult,
                            accum_out=acc[:, i : i + 1],
                        )
                    else:
                        nc.vector.memset(acc[:, i : i + 1], 0.5)
                else:
                    acc2 = acc  # per-chunk partials combined below
                    for c in range(NCHUNK):
                        m0 = c * CHUNK
                        pt = ppool.tile([128, CHUNK], F32, tag="ps")
                        for j in range(CHUNK // MM):
                            nc.tensor.matmul(
                                pt[:, j * MM : (j + 1) * MM],
                                lhsT,
                                Bm[g : g + KAUG, m0 + j * MM : m0 + (j + 1) * MM],
                            )
                        wt = wpool.tile([128, CHUNK], w_dtype, tag="w")
                        nc.scalar.activation(wt[:], pt[:], AF.Exp)
                        sc = tpool.tile([128, CHUNK], w_dtype, tag="sc")
                        nc.vector.scalar_tensor_tensor(
                            sc[:],
                            wt[:],
                            0.0,
                            s_bc[:, m0 : m0 + CHUNK],
                            OP.bypass,
                            OP.mult,
                            accum_out=accp[:, i * NCHUNK + c : i * NCHUNK + c + 1],
                        )

            # ---------------- clip + store ---------------------------------
            if not WIDE_STT:
                nc.vector.tensor_add(acc[:], accp[:, 0::NCHUNK], accp[:, 1::NCHUNK])
            res = cpool.tile([128, NB], F32)
            nc.vector.tensor_scalar(
                res[:],
                acc[:],
                MIN_DEPTH,
                MAX_DEPTH,
                OP.max,
                OP.min,
            )
            nc.sync.dma_start(out.rearrange("(i p) -> p i", p=128), res[:])

    nc.compile()
    return nc


_cache = {}


def _get_program(key, gw0, gw1, gw2, cb, w_dtype, repeat=1):
    key = key + (repeat,)
    if key not in _cache:
        _cache[key] = build_program(gw0, gw1, gw2, cb, w_dtype, repeat=repeat)
    return _cache[key]


def kernel(pred_depth, ray_3d, conv_w, conv_b, global_scale, repeat=1):
    pred_depth = np.asarray(pred_depth, np.float32)
    ray_3d = np.asarray(ray_3d, np.float32)
    g = float(np.asarray(global_scale).reshape(-1)[0])
    w = np.asarray(conv_w, np.float32).reshape(-1)
    cb = float(np.asarray(conv_b).reshape(-1)[0])
    gw0, gw1, gw2 = float(w[0] * g), float(w[1] * g), float(w[2] * g)

    nc = _get_program((gw0, gw1, gw2, cb), gw0, gw1, gw2, cb, FP16, repeat=repeat)

    in_maps = []
    for b in range(B):
        pp = np.zeros(N + 2, np.float32)
        pp[1 : N + 1] = pred_depth[b]
        in_maps.append(
            {
                "pred_pad": pp,
                "rayT": np.ascontiguousarray(ray_3d[b].T),
            }
        )
    res = _run_with_retry(nc, in_maps)
    out = np.stack([res.results[b]["out"] for b in range(B)]).astype(np.float32)
    return out


def _run_with_retry(nc, in_maps, tries=3):
    # The shared axon device occasionally reports a transient
    # NRT_EXEC_UNIT_UNRECOVERABLE after a prior process crashed; it
    # recovers within ~20s. Retry rather than failing the whole call.
    import time as _time

    for attempt in range(tries):
        try:
            return run_bass_kernel_spmd(nc, in_maps, core_ids=list(range(B)))
        except Exception:
            if attempt == tries - 1:
                raise
            _time.sleep(25)

